# revision 6
# baseline (speedup 1.0000x reference)
"""Distributed multi-head attention kernel for one TRN2 chip (8 NeuronCores).

Problem: b=2, n=2048, dim=1024, heads=16, hd=64.
  qkv = x @ Wqkv.T  (qkv-major split) -> RoPE(q,k) -> softmax(q k^T/8) v
  -> merge heads -> @ Wproj.T + bproj

Sharding: each core owns 2 heads (of 16) for BOTH batches. QKV projection,
RoPE and attention are fully head-local. Four 8-way AllToAlls (one per
batch x token-half, 128-token slices per core) redistribute attention
outputs head-major -> token-major so the tail exposes only the last
quarter of collective+projection. Core c outputs tokens
{1024h+128c : +128} for h in {0,1} of each batch; host reassembles.

v2 (vs baseline): qT/kT are computed DIRECTLY in [d, token] layout by
making the weight slices stationary and streaming xT (N=512) -- no PE
transposes. RoPE runs on DVE straight out of PSUM (4 partition-shifted
shift-mults + cos-mult + add per tensor per 512-token block) against
host-prepped cosT/snegT [128, 2048]. v is computed separately with
x-tiles stationary (N=128) landing directly in [tok, d] for AV. The
denominator broadcast uses one K=2 matmul (E-matrix) per (b, qq) instead
of two K=1 matmuls. ScalarE runs exp only; weight/sincos loads issue on
the gpsimd queue; all PSUM->SBUF copies are on DVE.

Per-core inputs (see make_in_maps), pre-cast/pre-transposed on host:
  x        [1024, 4096] bf16  x^T: channels x flat tokens
  wqk      [1024, 256]  bf16  per-ct [q|k] f-tiles? no: full (Wq|Wk rows for
                               my heads)^T, column blocks used per ct slice
  wv       [1024, 128]  bf16  (Wv rows for my heads)^T
  wproj    [1024, 1024] bf16  Wproj^T: [d', f]
  bproj    [1, 1024]    f32
  cosT     [128, 2048]  bf16  cos[t, d] rows (h*64+d), dup per head
  snegT    [128, 2048]  bf16  -sin[t,d] rows d<32; +sin[t,d] rows d>=32
  out      [512, 1024]  f32   rows = [b0h0, b0h1, b1h0, b1h1] x 128 tokens
"""

import os
import numpy as np

NUM_CORES = 8
B, N, DIM, NH, HD = 2, 2048, 1024, 16, 64
T = B * N                 # 4096 flat tokens
HPC = NH // NUM_CORES     # 2 heads per core
P = 128
CT = DIM // P             # 8 channel tiles
QW = HPC * HD             # 128 (q or k feature rows per core)
QB = 512                  # attention q-block width
TTH = N // P              # 16 token tiles per batch
NB = N // QB              # 4 q-blocks per batch
SL = P                    # 128 output tokens per (batch, half) slice

_CACHE = {}


def _build_nc():
    from concourse import bacc, mybir, tile

    f32 = mybir.dt.float32
    bf16 = mybir.dt.bfloat16
    Exp = mybir.ActivationFunctionType.Exp
    mult = mybir.AluOpType.mult
    add = mybir.AluOpType.add

    nc = bacc.Bacc("TRN2", target_bir_lowering=False, debug=False,
                   num_devices=NUM_CORES)

    x_d = nc.dram_tensor("x", [DIM, T], bf16, kind="ExternalInput")
    wqk_d = nc.dram_tensor("wqk", [DIM, 2 * QW], bf16, kind="ExternalInput")
    wv_d = nc.dram_tensor("wv", [DIM, QW], bf16, kind="ExternalInput")
    wproj_d = nc.dram_tensor("wproj", [DIM, DIM], bf16, kind="ExternalInput")
    bproj_d = nc.dram_tensor("bproj", [1, DIM], f32, kind="ExternalInput")
    cosT_d = nc.dram_tensor("cosT", [P, N], bf16, kind="ExternalInput")
    snegT_d = nc.dram_tensor("snegT", [P, N], bf16, kind="ExternalInput")
    out_d = nc.dram_tensor("out", [2 * B * SL, DIM], f32, kind="ExternalOutput")
    # one A2A tensor per (batch, token-half): 8 blocks of [128 d', 128 t]
    a2a_in = [[nc.dram_tensor(f"a2a_in{b}_{h}", [NUM_CORES * P, SL], bf16)
               for h in range(2)] for b in range(B)]
    a2a_out = [[nc.dram_tensor(f"a2a_out{b}_{h}", [NUM_CORES * P, SL], bf16)
                for h in range(2)] for b in range(B)]

    with tile.TileContext(nc) as tc:
        with (
            tc.tile_pool(name="persist", bufs=1) as pers,
            tc.tile_pool(name="work", bufs=3) as wp,
            tc.tile_pool(name="expp", bufs=4) as ep,
            tc.tile_pool(name="psQ", bufs=1, space="PSUM") as psQ,   # qk blocks
            tc.tile_pool(name="psM", bufs=1, space="PSUM") as psM,   # v/proj/bc
            tc.tile_pool(name="psS", bufs=2, space="PSUM") as psS,   # scores
            tc.tile_pool(name="psV", bufs=1, space="PSUM") as psV,   # av accum
        ):
            # ---------------- persistent SBUF ----------------
            wqkT = pers.tile([P, CT * 2 * QW], bf16)    # ct-block: [128c, 256f]
            wvT = pers.tile([P, CT * QW], bf16)         # ct-block: [128c, 128f]
            wprojT = pers.tile([P, CT * DIM], bf16)     # dt-block: [128d', 1024f]
            xT = pers.tile([P, CT * T], bf16)           # ct-block: [128c, 4096t]
            qT = pers.tile([P, T], bf16)                # [d(2 heads), flat t]
            kT = pers.tile([P, T], bf16)
            v_sb = pers.tile([P, HPC * (T // P) * 65], bf16)
            aoT = pers.tile([P, T], bf16)               # [d', flat t]
            aoTr = pers.tile([P, 2 * B * NUM_CORES * SL], bf16)
            cosT = pers.tile([P, N], bf16)
            snegT = pers.tile([P, N], bf16)
            ones_col = pers.tile([1, P], bf16)
            bias_bf = pers.tile([1, DIM], bf16)

            nc.vector.memset(ones_col, 1.0)
            # ones columns for the AV denominator (col 64 of each v block)
            vv_all = v_sb.rearrange("p (h t e) -> p h t e", h=HPC, t=T // P)
            nc.vector.memset(vv_all[:, :, :, 64:65], 1.0)

            # ---------------- prep loads ----------------
            # gpsimd queue: weights + rope tables + bias (order of need);
            # sync queue: xT in consumption order, one descriptor per
            # 512-token block covering all 8 ct slices.
            for ct in range(CT):
                nc.gpsimd.dma_start(wqkT[:, 2 * QW * ct:2 * QW * (ct + 1)],
                                    wqk_d[P * ct:P * (ct + 1), :])
            nc.gpsimd.dma_start(cosT, cosT_d.ap())
            nc.gpsimd.dma_start(snegT, snegT_d.ap())
            for ct in range(CT):
                nc.gpsimd.dma_start(wvT[:, QW * ct:QW * (ct + 1)],
                                    wv_d[P * ct:P * (ct + 1), :])
            bt = wp.tile([1, DIM], f32, tag="bload", bufs=1)
            nc.gpsimd.dma_start(bt, bproj_d[:, :])
            nc.vector.tensor_copy(bias_bf, bt)
            for ct in range(CT):
                nc.gpsimd.dma_start(wprojT[:, DIM * ct:DIM * (ct + 1)],
                                    wproj_d[P * ct:P * (ct + 1), :])

            xTv = xT.rearrange("p (ct t) -> p ct t", ct=CT)
            xdv = x_d.ap().rearrange("(ct p) t -> p ct t", p=P)
            for blk in range(T // QB):      # b0 blocks then b1 blocks
                nc.sync.dma_start(xTv[:, :, QB * blk:QB * (blk + 1)],
                                  xdv[:, :, QB * blk:QB * (blk + 1)])

            # ---------------- emitters ----------------
            def emit_qk_half(b, blk, which):
                """One 512-token block of qT or kT: 8 matmuls + RoPE on DVE.

                which: 0 = q, 1 = k. Weight f-tile stationary, xT moving;
                output lands in PSUM as [d(2 heads), 512 t]; RoPE reads the
                PSUM tile directly and writes bf16 qT/kT."""
                base = N * b + QB * blk
                qp = psQ.tile([P, QB], f32, tag="qk", name="qp")
                for ct in range(CT):
                    nc.tensor.matmul(
                        qp,
                        wqkT[:, 2 * QW * ct + QW * which:
                             2 * QW * ct + QW * (which + 1)],
                        xT[:, T * ct + base:T * ct + base + QB],
                        start=(ct == 0), stop=(ct == CT - 1))
                pos = QB * blk
                cs = cosT[:, pos:pos + QB]
                sn = snegT[:, pos:pos + QB]
                t1 = wp.tile([P, QB], bf16, tag="t1")
                for h in range(HPC):
                    r = HD * h
                    nc.vector.tensor_tensor(
                        t1[r:r + 32], qp[r + 32:r + 64], sn[r:r + 32], mult)
                    nc.vector.tensor_tensor(
                        t1[r + 32:r + 64], qp[r:r + 32], sn[r + 32:r + 64],
                        mult)
                qc = wp.tile([P, QB], bf16, tag="qc")
                nc.vector.tensor_tensor(qc, qp, cs, mult)
                dst = qT if which == 0 else kT
                nc.vector.tensor_tensor(dst[:, base:base + QB], qc, t1, add)

            def emit_v_tile(b, tt):
                """v for one 128-token tile: x-tile stationary, wv moving."""
                ftt = TTH * b + tt
                vp = psM.tile([P, QB], f32, tag="mm", name="vp")
                for ct in range(CT):
                    nc.tensor.matmul(
                        vp[:, 0:QW],
                        xT[:, T * ct + P * ftt:T * ct + P * (ftt + 1)],
                        wvT[:, QW * ct:QW * (ct + 1)],
                        start=(ct == 0), stop=(ct == CT - 1))
                nc.vector.tensor_copy(
                    vv_all[:, :, ftt, 0:HD],
                    vp[:, 0:QW].rearrange("p (h d) -> p h d", h=HPC))

            def emit_scores_exp(b, qq, jt):
                """Scores + exp for one (q-block, j-tile); returns exp tile."""
                ftt = TTH * b + jt
                sp = psS.tile([P, HPC * QB], f32, tag="scores", name="sp")
                for h in range(HPC):
                    nc.tensor.matmul(
                        sp[:, QB * h:QB * (h + 1)],
                        kT[HD * h:HD * (h + 1), P * ftt:P * (ftt + 1)],
                        qT[HD * h:HD * (h + 1),
                           N * b + QB * qq:N * b + QB * (qq + 1)],
                        start=True, stop=True)
                et = ep.tile([P, HPC * QB], bf16, tag="expT", name="et")
                nc.scalar.activation(et, sp, Exp, scale=float(HD) ** -0.5)
                return et

            def emit_av(b, qq, jt, av, et):
                ftt = TTH * b + jt
                for h in range(HPC):
                    blk = (h * (T // P) + ftt) * 65
                    nc.tensor.matmul(av[h], v_sb[:, blk:blk + 65],
                                     et[:, QB * h:QB * (h + 1)],
                                     start=(jt == 0), stop=(jt == TTH - 1))

            def emit_avf(avp):
                avf = wp.tile([65, HPC * QB], f32, tag="avf", bufs=2, name="avf")
                nc.vector.tensor_copy(avf, avp)
                return avf

            def emit_norm_rest(b, qq, avf):
                """Reciprocal of the denominator row, gpsimd partition
                broadcast, normalize; then A2A staging of this q-block's
                4 sub-chunks."""
                sums = wp.tile([1, HPC * QB], f32, tag="sums", name="sums")
                nc.vector.tensor_copy(sums, avf[64:65, :])
                rc1 = wp.tile([1, HPC * QB], f32, tag="rc1", name="rc1")
                nc.vector.reciprocal_approx_fast(rc1, sums)
                rcb = wp.tile([64, HPC * QB], f32, tag="recip", bufs=2,
                              name="rcb")
                for h in range(HPC):
                    nc.gpsimd.partition_broadcast(
                        rcb[:, QB * h:QB * (h + 1)],
                        rc1[:, QB * h:QB * (h + 1)])
                for h in range(HPC):
                    nc.vector.tensor_tensor(
                        aoT[HD * h:HD * (h + 1),
                            N * b + QB * qq:N * b + QB * (qq + 1)],
                        avf[0:64, QB * h:QB * (h + 1)],
                        rcb[:, QB * h:QB * (h + 1)], mult)
                # stage sub-chunks 4*(qq%2)..+3 of a2a_in[b][qq//2]
                r0 = 4 * P * (qq % 2)
                a2i = a2a_in[b][qq // 2][r0:r0 + 4 * P].rearrange(
                    "(c p) t -> p c t", p=P)
                nc.sync.dma_start(
                    a2i, aoT[:, N * b + QB * qq:N * b + QB * (qq + 1)].rearrange(
                        "p (c t) -> p c t", c=4))

            def emit_a2a(b, h):
                nc.gpsimd.collective_compute(
                    "AllToAll", mybir.AluOpType.bypass,
                    replica_groups=[list(range(NUM_CORES))],
                    ins=[a2a_in[b][h].ap().opt()],
                    outs=[a2a_out[b][h].ap().opt()])

            def proj_pieces(b, h):
                """Fetch A2A(b, h) result and project my 128-token slice.
                Returns a list of closures (emission pieces)."""
                pieces = []
                rbase = NUM_CORES * SL * (2 * b + h)

                def fetch():
                    a2o = a2a_out[b][h].ap().rearrange("(c p) t -> p c t", p=P)
                    nc.sync.dma_start(
                        aoTr[:, rbase:rbase + NUM_CORES * SL].rearrange(
                            "p (c t) -> p c t", c=NUM_CORES), a2o)
                pieces.append(fetch)

                def fb_piece(fb):
                    def run():
                        pp = psM.tile([P, QB], f32, tag="mm", name="proj")
                        for dt in range(CT):
                            lo = rbase + SL * dt
                            nc.tensor.matmul(
                                pp, aoTr[:, lo:lo + P],
                                wprojT[:, DIM * dt + QB * fb:
                                       DIM * dt + QB * (fb + 1)],
                                start=(dt == 0), stop=False)
                        nc.tensor.matmul(pp, ones_col[:, 0:P],
                                         bias_bf[:, QB * fb:QB * (fb + 1)],
                                         start=False, stop=True)
                        ob = wp.tile([P, QB], f32, tag="ob", bufs=2, name="ob")
                        nc.vector.tensor_copy(ob, pp)
                        orow = 2 * SL * b + SL * h
                        nc.sync.dma_start(
                            out_d[orow:orow + SL, QB * fb:QB * (fb + 1)], ob)
                    return run
                pieces.append(fb_piece(0))
                pieces.append(fb_piece(1))
                return pieces

            # ---------------- main schedule ----------------
            # Work queue of background emission pieces consumed between
            # attention (scores/exp/av) steps so PE/ACT stay saturated.
            state = {"pend": None, "carry": None}

            def flush_carry():
                if state["carry"] is not None:
                    state["carry"]()
                    state["carry"] = None

            def set_carry(b, qq, avp, av, et):
                def fn():
                    emit_av(b, qq, TTH - 1, av, et)
                    state["pend"] = (b, qq, emit_avf(avp))
                state["carry"] = fn

            def flush_pend():
                if state["pend"] is not None:
                    emit_norm_rest(*state["pend"])
                    state["pend"] = None

            # Phase A: b0 qk/v emission pipelined with qq0 scores.
            # scores(0, 0, jt) needs kT block jt//4 and qT block 0; AV jt
            # needs v tile jt. Each block emits k, 2 v tiles (covering
            # k-rope latency), q, 2 v tiles, then the previous block's 4
            # score steps run.
            avp = psV.tile([65, HPC * QB], f32, tag="av", name="avp")
            av = [avp[:, QB * h:QB * (h + 1)] for h in range(HPC)]
            prev_et = None
            jnext = 0

            def qq0_steps(upto):
                nonlocal prev_et, jnext
                while jnext < upto:
                    jt = jnext
                    et = emit_scores_exp(0, 0, jt)
                    if jt >= 1:
                        emit_av(0, 0, jt - 1, av, prev_et)
                    prev_et = et
                    jnext += 1

            for blk in range(NB):
                emit_qk_half(0, blk, 1)
                emit_v_tile(0, 4 * blk)
                emit_v_tile(0, 4 * blk + 1)
                emit_qk_half(0, blk, 0)
                emit_v_tile(0, 4 * blk + 2)
                emit_v_tile(0, 4 * blk + 3)
                if blk >= 1:
                    qq0_steps(4 * blk)
            qq0_steps(TTH)
            set_carry(0, 0, avp, av, prev_et)

            # Background queue for phases B/C:
            #  - b1 qk blocks + v tiles (all consumed during phase B)
            #  - proj pieces (gated by A2As, emitted well after the trigger)
            bg = []
            for blk in range(NB):
                tts = list(range(4 * blk, 4 * blk + 4))
                bg.append(lambda blk=blk: emit_qk_half(1, blk, 1))
                bg.append(lambda tt=tts[0]: emit_v_tile(1, tt))
                bg.append(lambda tt=tts[1]: emit_v_tile(1, tt))
                bg.append(lambda blk=blk: emit_qk_half(1, blk, 0))
                bg.append(lambda tt=tts[2]: emit_v_tile(1, tt))
                bg.append(lambda tt=tts[3]: emit_v_tile(1, tt))

            def run_bg(n):
                for _ in range(n):
                    if bg:
                        bg.pop(0)()

            # Phase B: b0 qq1..3 (b1 qk/v interleaved, 2 pieces per slot);
            # Phase C: b1 qq0..3 (proj pieces interleaved).
            # A2A(b, h) fires after the pend-flush of qq=2h+1 (both q-blocks
            # staged); its proj pieces enter the queue one q-block later.
            qlist = [(0, qq) for qq in range(1, NB)] + \
                    [(1, qq) for qq in range(NB)]
            a2a_after = {(0, 2): (0, 0), (1, 0): (0, 1), (1, 2): (1, 0)}
            proj_at = {(0, 3): (0, 0), (1, 1): (0, 1), (1, 3): (1, 0)}
            for (b, qq) in qlist:
                avp = psV.tile([65, HPC * QB], f32, tag="av", name="avp")
                av = [avp[:, QB * h:QB * (h + 1)] for h in range(HPC)]
                prev_et = None
                if (b, qq) in proj_at:
                    bg.extend(proj_pieces(*proj_at[(b, qq)]))
                for jt in range(TTH):
                    et = emit_scores_exp(b, qq, jt)
                    if jt == 0:
                        flush_carry()
                    else:
                        emit_av(b, qq, jt - 1, av, prev_et)
                    prev_et = et
                    if jt == 2:
                        flush_pend()
                        if (b, qq) in a2a_after:
                            emit_a2a(*a2a_after[(b, qq)])
                    if jt % 3 == 2:
                        run_bg(2)
                set_carry(b, qq, avp, av, prev_et)
            flush_carry()
            flush_pend()
            run_bg(len(bg))
            emit_a2a(1, 1)
            for piece in proj_pieces(1, 1):
                piece()

    nc.compile()
    return nc


def _get_nc():
    if "nc" not in _CACHE:
        _CACHE["nc"] = _build_nc()
    return _CACHE["nc"]


def make_in_maps(x, Wqkv, Wproj, bproj, sin, cos):
    """Shard full (f32) inputs into per-core in_maps (pre-cast/transposed)."""
    import ml_dtypes
    bf16 = ml_dtypes.bfloat16
    xT = np.ascontiguousarray(
        np.asarray(x, np.float32).reshape(T, DIM).astype(bf16).T)
    Wqkv = np.asarray(Wqkv, np.float32)
    WprojT = np.ascontiguousarray(np.asarray(Wproj, np.float32).astype(bf16).T)
    bproj = np.asarray(bproj, np.float32).reshape(1, DIM)
    sin = np.asarray(sin, np.float32)      # [N, HD]
    cos = np.asarray(cos, np.float32)
    # cosT/snegT [128, N]: rows h*64+d; sneg rows d<32 = -sin[:, d],
    # rows d>=32 = +sin[:, d]
    cosT = np.empty((P, N), np.float32)
    snegT = np.empty((P, N), np.float32)
    for h in range(HPC):
        cosT[HD * h:HD * (h + 1)] = cos.T
        snegT[HD * h:HD * h + 32] = -sin.T[:32]
        snegT[HD * h + 32:HD * (h + 1)] = sin.T[32:]
    cosT = cosT.astype(bf16)
    snegT = snegT.astype(bf16)
    in_maps = []
    for c in range(NUM_CORES):
        r = P * c
        wq = Wqkv[r:r + P]                    # [128 f, 1024 c]
        wk = Wqkv[DIM + r:DIM + r + P]
        wv = Wqkv[2 * DIM + r:2 * DIM + r + P]
        wqk = np.concatenate([wq, wk], 0).astype(bf16)   # [256 f, 1024 c]
        in_maps.append({
            "x": xT,
            "wqk": np.ascontiguousarray(wqk.T),
            "wv": np.ascontiguousarray(wv.astype(bf16).T),
            "wproj": WprojT,
            "bproj": bproj,
            "cosT": cosT,
            "snegT": snegT,
        })
    return in_maps


def reassemble(outs):
    """outs[c] = [512, 1024] f32, rows [b0h0|b0h1|b1h0|b1h1] x 128 t."""
    out = np.empty((B, N, DIM), np.float32)
    for c in range(NUM_CORES):
        o = outs[c]
        for b in range(B):
            for h in range(2):
                t0 = N // 2 * h + SL * c
                out[b, t0:t0 + SL] = o[2 * SL * b + SL * h:
                                       2 * SL * b + SL * (h + 1)]
    return out


def kernel(x, Wqkv, Wproj, bproj, sin, cos):
    from concourse.bass_utils import run_bass_kernel_spmd

    nc = _get_nc()
    in_maps = make_in_maps(x, Wqkv, Wproj, bproj, sin, cos)
    trace = bool(int(os.environ.get("KERNEL_TRACE", "0")))
    res = run_bass_kernel_spmd(nc, in_maps, core_ids=list(range(NUM_CORES)),
                               trace=trace)
    _CACHE["last_result"] = res
    return reassemble([res.results[c]["out"] for c in range(NUM_CORES)])
